# revision 3
# baseline (speedup 1.0000x reference)
"""Distributed KNN retrieval kernel for Trainium2 (8 NeuronCores).

Strategy (standard distributed-KNN):
  - Shard the memory bank (mem_keys/mem_values + metadata) across 8 cores
    along the memory axis (8192 memories per core).
  - Each core: scores = (q @ k_shard.T) * (retention*importance*freq / ||k||)
    via fp32 PE matmul with PSUM accumulation, then hardware top-8
    (InstMax/InstMaxIndex) per query, then indirect-DMA gather of its 8
    candidate value rows per query.
  - AllGather the 8*8 candidate scores per query; every core computes the
    global top-8 threshold + softmax normalizer, weights its own surviving
    candidates, and emits a partial weighted combine.
  - ReduceScatter sums the partials; each core outputs a 32-query slice of
    the combined output. Host concatenates the slices.

The key matrix is fed pre-transposed ([D, M_shard], a pure layout change
done on the host) so the contraction dim lands on SBUF partitions.
"""

import sys

for p in ("/opt/trn_rl_repo", "/opt/pypackages", "/root/.axon_site"):
    if p not in sys.path:
        sys.path.insert(0, p)

import math
import numpy as np

import concourse.bass as bass
import concourse.bacc as bacc
import concourse.mybir as mybir
import concourse.tile as tile
from concourse.bass_utils import run_bass_kernel_spmd

N_CORES = 8
B = 256  # queries
D = 1024  # feature dim
M = 65536  # memory bank size
MC = M // N_CORES  # memories per core (8192)
K = 8  # top_k
NB = B // 128  # query partition tiles (2)
ND = D // 128  # contraction chunks (8)
MCHUNK = 512  # moving free dim per matmul
NCH = MC // MCHUNK  # m-chunks per core (16)
CUR_TIME = 1000.0
DECAY_RATE = 0.999
DECAY_EPS = 1e-8

F32 = mybir.dt.float32
I32 = mybir.dt.int32
U32 = mybir.dt.uint32
AF = mybir.ActivationFunctionType
ALU = mybir.AluOpType


def _build_program():
    nc = bacc.Bacc("TRN2", target_bir_lowering=False, debug=False, num_devices=N_CORES)

    # Per-core inputs
    qT = nc.dram_tensor("qT", [D, B], F32, kind="ExternalInput").ap()
    kT = nc.dram_tensor("kT", [D, MC], F32, kind="ExternalInput").ap()
    vals = nc.dram_tensor("vals", [MC, D], F32, kind="ExternalInput").ap()
    imp = nc.dram_tensor("imp", [MC], F32, kind="ExternalInput").ap()
    at = nc.dram_tensor("at", [MC], I32, kind="ExternalInput").ap()
    cnt = nc.dram_tensor("cnt", [MC], I32, kind="ExternalInput").ap()

    # Per-core outputs
    comb = nc.dram_tensor("comb", [B // N_CORES, D], F32, kind="ExternalOutput").ap()
    conf = nc.dram_tensor("conf", [B], F32, kind="ExternalOutput").ap()

    with tile.TileContext(nc) as tc:
        with (
            tc.tile_pool(name="const", bufs=1) as constp,
            tc.tile_pool(name="meta", bufs=1) as metap,
            tc.tile_pool(name="kt", bufs=3) as ktp,
            tc.tile_pool(name="sq", bufs=3) as sqp,
            tc.tile_pool(name="acc", bufs=2) as accp,
            tc.tile_pool(name="wb", bufs=2) as wbp,
            tc.tile_pool(name="scores", bufs=1) as scoresp,
            tc.tile_pool(name="small", bufs=1) as smallp,
            tc.tile_pool(name="vg", bufs=3) as vgp,
            tc.tile_pool(name="pc", bufs=1) as pcp,
            tc.tile_pool(name="psum", bufs=4, space="PSUM") as psump,
            tc.tile_pool(name="psn", bufs=2, space="PSUM") as psnp,
            tc.tile_pool(name="dram", bufs=1, space="DRAM") as dramp,
        ):
            # ---------------- constants / loads ----------------
            ones_col = constp.tile([128, 1], F32)
            nc.vector.memset(ones_col[:], 1.0)

            qt_sb = constp.tile([128, ND, B], F32)  # [p, j, b] : d = 128*j + p
            nc.sync.dma_start(qt_sb[:], qT.rearrange("(j p) b -> p j b", p=128))

            # ---------------- metadata -> meta (partition layout) ------------
            # m = 64*p + f
            imp_t = metap.tile([128, MC // 128], F32, tag="m_imp")
            at_i = metap.tile([128, MC // 128], I32, tag="m_at")
            cnt_i = metap.tile([128, MC // 128], I32, tag="m_cnt")
            nc.sync.dma_start(imp_t[:], imp.rearrange("(p f) -> p f", p=128))
            nc.sync.dma_start(at_i[:], at.rearrange("(p f) -> p f", p=128))
            nc.sync.dma_start(cnt_i[:], cnt.rearrange("(p f) -> p f", p=128))

            dt_f = metap.tile([128, MC // 128], F32, tag="m_dt")
            nc.vector.tensor_copy(out=dt_f[:], in_=at_i[:])  # i32 -> f32
            # dt = CUR_TIME - at  (as -1*at + CUR_TIME)
            nc.vector.tensor_scalar(
                out=dt_f[:], in0=dt_f[:], scalar1=-1.0, scalar2=CUR_TIME,
                op0=ALU.mult, op1=ALU.add,
            )
            ret_t = metap.tile([128, MC // 128], F32, tag="m_ret")
            nc.scalar.activation(
                out=ret_t[:], in_=dt_f[:], func=AF.Exp, scale=float(math.log(DECAY_RATE))
            )
            cnt_f = metap.tile([128, MC // 128], F32, tag="m_cntf")
            nc.vector.tensor_copy(out=cnt_f[:], in_=cnt_i[:])
            fb_t = metap.tile([128, MC // 128], F32, tag="m_fb")
            nc.scalar.activation(out=fb_t[:], in_=cnt_f[:], func=AF.Ln, bias=1.0)
            meta_t = metap.tile([128, MC // 128], F32, tag="m_meta")
            nc.vector.tensor_tensor(out=meta_t[:], in0=ret_t[:], in1=imp_t[:], op=ALU.mult)
            nc.vector.tensor_tensor(out=meta_t[:], in0=meta_t[:], in1=fb_t[:], op=ALU.mult)

            # bounce meta to a [1, MC] row layout
            d_meta = dramp.tile([MC], F32)
            nc.sync.dma_start(d_meta[:].rearrange("(p f) -> p f", p=128), meta_t[:])

            # ---------------- query norms -> qinv columns ----------------
            accq = accp.tile([128, B], F32, tag="accq")
            sqq = sqp.tile([128, B], F32, tag="sqq")
            for j in range(ND):
                tgt = accq if j == 0 else sqq
                nc.scalar.activation(out=tgt[:], in_=qt_sb[:, j, :], func=AF.Square)
                if j > 0:
                    nc.vector.tensor_tensor(out=accq[:], in0=accq[:], in1=sqq[:], op=ALU.add)
            psq = psnp.tile([1, B], F32, tag="psq")
            nc.tensor.matmul(psq[:], ones_col[:], accq[:], start=True, stop=True)
            qn_row = smallp.tile([1, B], F32, tag="qn_row")
            nc.scalar.activation(out=qn_row[:], in_=psq[:], func=AF.Sqrt)
            nc.vector.tensor_scalar_max(qn_row[:], qn_row[:], DECAY_EPS)
            nc.vector.reciprocal(out=qn_row[:], in_=qn_row[:])
            d_qinv = dramp.tile([B], F32)
            nc.sync.dma_start(d_qinv[:].unsqueeze(0), qn_row[:])
            qinv_col = smallp.tile([128, NB], F32, tag="qinv_col")
            for t in range(NB):
                nc.sync.dma_start(
                    qinv_col[:, t : t + 1], d_qinv[t * 128 : (t + 1) * 128].unsqueeze(-1)
                )

            # ---------------- main loop: scores + k-norms ----------------
            scores_sb = [
                scoresp.tile([128, MC], F32, tag=f"sc{t}", name=f"scores{t}")
                for t in range(NB)
            ]

            for ci in range(NCH):
                kt_t = ktp.tile([128, ND, MCHUNK], F32, tag="kt")
                nc.sync.dma_start(
                    kt_t[:],
                    kT.rearrange("(j p) (c n) -> c p j n", p=128, n=MCHUNK)[ci],
                )

                # squared-column accumulation for ||k||^2
                acck = accp.tile([128, MCHUNK], F32, tag="acck")
                for j in range(ND):
                    tgt = acck if j == 0 else sqp.tile([128, MCHUNK], F32, tag="sqk")
                    nc.scalar.activation(out=tgt[:], in_=kt_t[:, j, :], func=AF.Square)
                    if j > 0:
                        nc.vector.tensor_tensor(out=acck[:], in0=acck[:], in1=tgt[:], op=ALU.add)
                psn = psnp.tile([1, MCHUNK], F32, tag="psn")
                nc.tensor.matmul(psn[:], ones_col[:], acck[:], start=True, stop=True)

                # w = meta / max(sqrt(ss), eps)
                mrow = wbp.tile([1, MCHUNK], F32, tag="mrow", name="mrow")
                nc.sync.dma_start(
                    mrow[:], d_meta[ci * MCHUNK : (ci + 1) * MCHUNK].unsqueeze(0)
                )
                wr = wbp.tile([1, MCHUNK], F32, tag="wr", name="wr")
                nc.scalar.activation(out=wr[:], in_=psn[:], func=AF.Sqrt)
                nc.vector.tensor_scalar_max(wr[:], wr[:], DECAY_EPS)
                nc.vector.reciprocal(out=wr[:], in_=wr[:])
                nc.vector.tensor_tensor(out=wr[:], in0=wr[:], in1=mrow[:], op=ALU.mult)
                w_bc = wbp.tile([128, MCHUNK], F32, tag="w_bc")
                nc.gpsimd.partition_broadcast(w_bc[:], wr[:])

                # scores matmuls
                for t in range(NB):
                    ps = psump.tile([128, MCHUNK], F32, tag="ps")
                    for j in range(ND):
                        nc.tensor.matmul(
                            ps[:],
                            qt_sb[:, j, t * 128 : (t + 1) * 128],
                            kt_t[:, j, :],
                            start=(j == 0),
                            stop=(j == ND - 1),
                        )
                    nc.vector.tensor_tensor(
                        out=scores_sb[t][:, ci * MCHUNK : (ci + 1) * MCHUNK],
                        in0=ps[:],
                        in1=w_bc[:],
                        op=ALU.mult,
                    )

            # ---------------- local top-8 + value gather ----------------
            t8 = [smallp.tile([128, K], F32, tag=f"t8_{t}", name=f"t8_{t}") for t in range(NB)]
            i8 = [smallp.tile([128, K], U32, tag=f"i8_{t}", name=f"i8_{t}") for t in range(NB)]
            ag_in = dramp.tile([B, K], F32)
            for t in range(NB):
                nc.vector.max(out=t8[t][:], in_=scores_sb[t][:])
                nc.vector.max_index(out=i8[t][:], in_max=t8[t][:], in_values=scores_sb[t][:])
                nc.sync.dma_start(ag_in[t * 128 : (t + 1) * 128, :], t8[t][:])

            ag_out = dramp.tile([N_CORES * B, K], F32)
            nc.gpsimd.collective_compute(
                "AllGather",
                ALU.bypass,
                replica_groups=[list(range(N_CORES))],
                ins=[ag_in.opt()],
                outs=[ag_out.opt()],
            )

            # ---------------- global stage ----------------
            rs_in = dramp.tile([B, D], F32)
            for t in range(NB):
                g = smallp.tile([128, N_CORES, K], F32, tag="g")
                nc.sync.dma_start(
                    g[:],
                    ag_out[:].rearrange("(c t p) k -> t p c k", c=N_CORES, p=128)[t],
                )
                qv = qinv_col[:, t : t + 1]
                gf = g[:].rearrange("p c k -> p (c k)")
                nc.vector.tensor_scalar(
                    out=gf, in0=gf, scalar1=qv, scalar2=None, op0=ALU.mult
                )
                g8 = smallp.tile([128, K], F32, tag="g8")
                nc.vector.max(out=g8[:], in_=gf)
                m1 = g8[:, 0:1]
                thr = g8[:, K - 1 : K]
                negm1 = smallp.tile([128, 1], F32, tag="negm1")
                nc.vector.tensor_scalar(
                    out=negm1[:], in0=m1, scalar1=-1.0, scalar2=None, op0=ALU.mult
                )
                e8 = smallp.tile([128, K], F32, tag="e8")
                zsum = smallp.tile([128, 1], F32, tag="zsum")
                nc.scalar.activation(
                    out=e8[:], in_=g8[:], func=AF.Exp, bias=negm1[:], accum_out=zsum[:]
                )
                zinv = smallp.tile([128, 1], F32, tag="zinv")
                nc.vector.reciprocal(out=zinv[:], in_=zsum[:])
                nc.sync.dma_start(conf[t * 128 : (t + 1) * 128].unsqueeze(-1), zinv[:])

                # own candidate weights
                so = smallp.tile([128, K], F32, tag="so")
                nc.vector.tensor_scalar(
                    out=so[:], in0=t8[t][:], scalar1=qv, scalar2=None, op0=ALU.mult
                )
                msk = smallp.tile([128, K], F32, tag="msk")
                nc.vector.tensor_scalar(
                    out=msk[:], in0=so[:], scalar1=thr, scalar2=None, op0=ALU.is_ge
                )
                eo = smallp.tile([128, K], F32, tag="eo")
                nc.scalar.activation(out=eo[:], in_=so[:], func=AF.Exp, bias=negm1[:])
                wloc = smallp.tile([128, K], F32, tag="wloc")
                nc.vector.tensor_tensor(out=wloc[:], in0=eo[:], in1=msk[:], op=ALU.mult)
                nc.vector.tensor_scalar(
                    out=wloc[:], in0=wloc[:], scalar1=zinv[:], scalar2=None, op0=ALU.mult
                )

                # partial combine: pc = sum_k wloc[:, k] * vals[i8[:, k], :]
                pc = pcp.tile([128, D], F32, tag="pc")
                for k in range(K):
                    vg = vgp.tile([128, D], F32, tag="vg")
                    nc.gpsimd.indirect_dma_start(
                        out=vg[:],
                        out_offset=None,
                        in_=vals[:],
                        in_offset=bass.IndirectOffsetOnAxis(ap=i8[t][:, k : k + 1], axis=0),
                    )
                    if k == 0:
                        nc.vector.tensor_scalar(
                            out=pc[:], in0=vg[:], scalar1=wloc[:, 0:1], scalar2=None,
                            op0=ALU.mult,
                        )
                    else:
                        nc.vector.scalar_tensor_tensor(
                            out=pc[:], in0=vg[:], scalar=wloc[:, k : k + 1], in1=pc[:],
                            op0=ALU.mult, op1=ALU.add,
                        )
                nc.sync.dma_start(rs_in[t * 128 : (t + 1) * 128, :], pc[:])

            rs_out = dramp.tile([B // N_CORES, D], F32)
            nc.gpsimd.collective_compute(
                "ReduceScatter",
                ALU.add,
                replica_groups=[list(range(N_CORES))],
                ins=[rs_in.opt()],
                outs=[rs_out.opt()],
            )
            nc.sync.dma_start(comb[:], rs_out[:])

    nc.compile()
    return nc


_PROGRAM = None


def _get_program():
    global _PROGRAM
    if _PROGRAM is None:
        _PROGRAM = _build_program()
    return _PROGRAM


def run_on_hw(in_maps, trace=False):
    nc = _get_program()
    return run_bass_kernel_spmd(
        nc, in_maps, core_ids=list(range(N_CORES)), trace=trace
    )


def make_in_maps(query, mem_keys, mem_values, importance, access_times, access_counts):
    query = np.asarray(query, dtype=np.float32)
    mem_keys = np.asarray(mem_keys, dtype=np.float32)
    mem_values = np.asarray(mem_values, dtype=np.float32)
    importance = np.asarray(importance, dtype=np.float32)
    access_times = np.asarray(access_times, dtype=np.int32)
    access_counts = np.asarray(access_counts, dtype=np.int32)

    qT_np = np.ascontiguousarray(query.T)
    in_maps = []
    for c in range(N_CORES):
        sl = slice(c * MC, (c + 1) * MC)
        in_maps.append(
            {
                "qT": qT_np,
                "kT": np.ascontiguousarray(mem_keys[sl].T),
                "vals": np.ascontiguousarray(mem_values[sl]),
                "imp": importance[sl],
                "at": access_times[sl],
                "cnt": access_counts[sl],
            }
        )
    return in_maps


def kernel(
    query,
    mem_keys,
    mem_values,
    importance,
    access_times,
    access_counts,
    current_time,
    top_k,
    _trace=False,
    _results_out=None,
):
    assert int(current_time) == 1000 and int(top_k) == 8
    in_maps = make_in_maps(
        query, mem_keys, mem_values, importance, access_times, access_counts
    )
    res = run_on_hw(in_maps, trace=_trace)
    if _results_out is not None:
        _results_out.append(res)
    combined = np.concatenate(
        [res.results[c]["comb"] for c in range(N_CORES)], axis=0
    )
    confidence = res.results[0]["conf"]
    return combined, confidence


# revision 5
# speedup vs baseline: 1.1726x; 1.1726x over previous
"""Distributed KNN retrieval kernel for Trainium2 (8 NeuronCores).

Strategy (standard distributed-KNN):
  - Shard the memory bank (mem_keys/mem_values + metadata) across 8 cores
    along the memory axis (8192 memories per core).
  - Each core: scores = (q @ k_shard.T) * (retention*importance*freq / ||k||)
    with the fp32 matmul decomposed into 3 bf16 matmuls (hi/lo split, done
    on the host as an input re-encoding; error ~2^-16 relative, far below
    the top-8 ranking margins), then hardware top-8 (InstMax/InstMaxIndex)
    per query, then indirect-DMA gather of its 8 candidate value rows.
  - AllGather the 8*8 candidate scores per query; every core computes the
    global top-8 threshold + softmax normalizer, weights its own surviving
    candidates, and emits a partial weighted combine.
  - ReduceScatter sums the partials; each core outputs a 32-query slice of
    the combined output. Host concatenates the slices.

Key norms use a pre-squared fp16 copy of the keys (ones-vector matmul
reduces over the contraction partitions), avoiding elementwise squares on
the vector engine. All transcendentals (decay exp, log1p, 1/sqrt via
exp(-0.5 ln), softmax) run on the scalar engine.
"""

import sys

for p in ("/opt/trn_rl_repo", "/opt/pypackages", "/root/.axon_site"):
    if p not in sys.path:
        sys.path.insert(0, p)

import math
import numpy as np
import ml_dtypes

import concourse.bass as bass
import concourse.bacc as bacc
import concourse.mybir as mybir
import concourse.tile as tile
from concourse.bass_utils import run_bass_kernel_spmd

N_CORES = 8
B = 256  # queries
D = 1024  # feature dim
M = 65536  # memory bank size
MC = M // N_CORES  # memories per core (8192)
K = 8  # top_k
NB = B // 128  # query partition tiles (2)
ND = D // 128  # contraction chunks (8)
MCHUNK = 512  # moving free dim per matmul
NCH = MC // MCHUNK  # m-chunks per core (16)
CUR_TIME = 1000.0
DECAY_RATE = 0.999
DECAY_EPS = 1e-8

F32 = mybir.dt.float32
F16 = mybir.dt.float16
BF16 = mybir.dt.bfloat16
I32 = mybir.dt.int32
U32 = mybir.dt.uint32
AF = mybir.ActivationFunctionType
ALU = mybir.AluOpType


def _build_program():
    nc = bacc.Bacc("TRN2", target_bir_lowering=False, debug=False, num_devices=N_CORES)

    # Per-core inputs
    qT = nc.dram_tensor("qT", [D, B], F32, kind="ExternalInput").ap()
    qh = nc.dram_tensor("qh", [D, B], BF16, kind="ExternalInput").ap()
    ql = nc.dram_tensor("ql", [D, B], BF16, kind="ExternalInput").ap()
    kh = nc.dram_tensor("kh", [D, MC], BF16, kind="ExternalInput").ap()
    kl = nc.dram_tensor("kl", [D, MC], BF16, kind="ExternalInput").ap()
    ksq = nc.dram_tensor("ksq", [D, MC], F16, kind="ExternalInput").ap()
    vals = nc.dram_tensor("vals", [MC, D], F32, kind="ExternalInput").ap()
    imp = nc.dram_tensor("imp", [MC], F32, kind="ExternalInput").ap()
    at = nc.dram_tensor("at", [MC], I32, kind="ExternalInput").ap()
    cnt = nc.dram_tensor("cnt", [MC], I32, kind="ExternalInput").ap()

    # Per-core outputs
    comb = nc.dram_tensor("comb", [B // N_CORES, D], F32, kind="ExternalOutput").ap()
    conf = nc.dram_tensor("conf", [B], F32, kind="ExternalOutput").ap()

    with tile.TileContext(nc) as tc:
        with (
            tc.tile_pool(name="const", bufs=1) as constp,
            tc.tile_pool(name="meta", bufs=1) as metap,
            tc.tile_pool(name="kt", bufs=2) as ktp,
            tc.tile_pool(name="acc", bufs=2) as accp,
            tc.tile_pool(name="wb", bufs=2) as wbp,
            tc.tile_pool(name="scores", bufs=1) as scoresp,
            tc.tile_pool(name="small", bufs=1) as smallp,
            tc.tile_pool(name="vg", bufs=10) as vgp,
            tc.tile_pool(name="pc", bufs=2) as pcp,
            tc.tile_pool(name="psum", bufs=4, space="PSUM") as psump,
            tc.tile_pool(name="psn", bufs=2, space="PSUM") as psnp,
            tc.tile_pool(name="dram", bufs=1, space="DRAM") as dramp,
        ):
            # ---------------- constants / query loads ----------------
            qh_sb = constp.tile([128, ND, B], BF16)  # [p, j, b] : d = 128*j + p
            ql_sb = constp.tile([128, ND, B], BF16)
            nc.sync.dma_start(qh_sb[:], qh.rearrange("(j p) b -> p j b", p=128))
            nc.sync.dma_start(ql_sb[:], ql.rearrange("(j p) b -> p j b", p=128))

            ones16 = constp.tile([128, 1], F16)
            nc.vector.memset(ones16[:], 1.0)

            qt_sb = constp.tile([128, ND, B], F32)
            nc.sync.dma_start(qt_sb[:], qT.rearrange("(j p) b -> p j b", p=128))

            # ---------------- metadata -> meta (partition layout) ------------
            # m = 64*p + f
            imp_t = metap.tile([128, MC // 128], F32, tag="m_imp")
            at_i = metap.tile([128, MC // 128], I32, tag="m_at")
            cnt_i = metap.tile([128, MC // 128], I32, tag="m_cnt")
            nc.sync.dma_start(imp_t[:], imp.rearrange("(p f) -> p f", p=128))
            nc.sync.dma_start(at_i[:], at.rearrange("(p f) -> p f", p=128))
            nc.sync.dma_start(cnt_i[:], cnt.rearrange("(p f) -> p f", p=128))

            dt_f = metap.tile([128, MC // 128], F32, tag="m_dt")
            nc.vector.tensor_copy(out=dt_f[:], in_=at_i[:])  # i32 -> f32
            # dt = CUR_TIME - at  (as -1*at + CUR_TIME)
            nc.vector.tensor_scalar(
                out=dt_f[:], in0=dt_f[:], scalar1=-1.0, scalar2=CUR_TIME,
                op0=ALU.mult, op1=ALU.add,
            )
            ret_t = metap.tile([128, MC // 128], F32, tag="m_ret")
            nc.scalar.activation(
                out=ret_t[:], in_=dt_f[:], func=AF.Exp, scale=float(math.log(DECAY_RATE))
            )
            cnt_f = metap.tile([128, MC // 128], F32, tag="m_cntf")
            nc.vector.tensor_copy(out=cnt_f[:], in_=cnt_i[:])
            fb_t = metap.tile([128, MC // 128], F32, tag="m_fb")
            nc.scalar.activation(out=fb_t[:], in_=cnt_f[:], func=AF.Ln, bias=1.0)
            meta_t = metap.tile([128, MC // 128], F32, tag="m_meta")
            nc.vector.tensor_tensor(out=meta_t[:], in0=ret_t[:], in1=imp_t[:], op=ALU.mult)
            nc.vector.tensor_tensor(out=meta_t[:], in0=meta_t[:], in1=fb_t[:], op=ALU.mult)

            # bounce meta to DRAM for [1, chunk] row loads
            d_meta = dramp.tile([MC], F32)
            nc.sync.dma_start(d_meta[:].rearrange("(p f) -> p f", p=128), meta_t[:])

            # ---------------- query norms -> qinv columns ----------------
            accq = accp.tile([128, B], F32, tag="accq")
            sqq = accp.tile([128, B], F32, tag="sqq")
            for j in range(ND):
                tgt = accq if j == 0 else sqq
                nc.scalar.activation(out=tgt[:], in_=qt_sb[:, j, :], func=AF.Square)
                if j > 0:
                    nc.vector.tensor_tensor(out=accq[:], in0=accq[:], in1=sqq[:], op=ALU.add)
            ones32 = constp.tile([128, 1], F32)
            nc.vector.memset(ones32[:], 1.0)
            psq = psnp.tile([1, B], F32, tag="psq")
            nc.tensor.matmul(psq[:], ones32[:], accq[:], start=True, stop=True)
            # qinv = 1/sqrt(ssq) = exp(-0.5*ln(ssq))
            qn_row = smallp.tile([1, B], F32, tag="qn_row")
            nc.scalar.activation(out=qn_row[:], in_=psq[:], func=AF.Ln)
            nc.scalar.activation(out=qn_row[:], in_=qn_row[:], func=AF.Exp, scale=-0.5)
            d_qinv = dramp.tile([B], F32)
            nc.sync.dma_start(d_qinv[:].unsqueeze(0), qn_row[:])
            qinv_col = smallp.tile([128, NB], F32, tag="qinv_col")
            for t in range(NB):
                nc.sync.dma_start(
                    qinv_col[:, t : t + 1], d_qinv[t * 128 : (t + 1) * 128].unsqueeze(-1)
                )

            # ---------------- main loop: scores + k-norms ----------------
            scores_sb = [
                scoresp.tile([128, MC], F32, tag=f"sc{t}", name=f"scores{t}")
                for t in range(NB)
            ]

            for ci in range(NCH):
                kh_t = ktp.tile([128, ND, MCHUNK], BF16, tag="kh", name="kh_t")
                kl_t = ktp.tile([128, ND, MCHUNK], BF16, tag="kl", name="kl_t")
                ks_t = ktp.tile([128, ND, MCHUNK], F16, tag="ks", name="ks_t")
                view = "(j p) (c n) -> c p j n"
                nc.sync.dma_start(kh_t[:], kh.rearrange(view, p=128, n=MCHUNK)[ci])
                nc.sync.dma_start(kl_t[:], kl.rearrange(view, p=128, n=MCHUNK)[ci])
                nc.sync.dma_start(ks_t[:], ksq.rearrange(view, p=128, n=MCHUNK)[ci])

                # ||k||^2 via ones-matmul over partitions, accumulated over j
                psn = psnp.tile([1, MCHUNK], F32, tag="psn")
                for j in range(ND):
                    nc.tensor.matmul(
                        psn[:], ones16[:], ks_t[:, j, :],
                        start=(j == 0), stop=(j == ND - 1),
                    )

                # w = meta * exp(-0.5*ln(ss))
                mrow = wbp.tile([1, MCHUNK], F32, tag="mrow", name="mrow")
                nc.sync.dma_start(
                    mrow[:], d_meta[ci * MCHUNK : (ci + 1) * MCHUNK].unsqueeze(0)
                )
                wr = wbp.tile([1, MCHUNK], F32, tag="wr", name="wr")
                nc.scalar.activation(out=wr[:], in_=psn[:], func=AF.Ln)
                nc.scalar.activation(out=wr[:], in_=wr[:], func=AF.Exp, scale=-0.5)
                nc.vector.tensor_tensor(out=wr[:], in0=wr[:], in1=mrow[:], op=ALU.mult)
                w_bc = wbp.tile([128, MCHUNK], F32, tag="w_bc")
                nc.gpsimd.partition_broadcast(w_bc[:], wr[:])

                # scores matmuls: 3 bf16 groups accumulate into one psum tile
                for t in range(NB):
                    ps = psump.tile([128, MCHUNK], F32, tag="ps")
                    groups = [(qh_sb, kh_t), (qh_sb, kl_t), (ql_sb, kh_t)]
                    n_mm = len(groups) * ND
                    i_mm = 0
                    for qsb, ktile in groups:
                        for j in range(ND):
                            nc.tensor.matmul(
                                ps[:],
                                qsb[:, j, t * 128 : (t + 1) * 128],
                                ktile[:, j, :],
                                start=(i_mm == 0),
                                stop=(i_mm == n_mm - 1),
                            )
                            i_mm += 1
                    nc.vector.tensor_tensor(
                        out=scores_sb[t][:, ci * MCHUNK : (ci + 1) * MCHUNK],
                        in0=ps[:],
                        in1=w_bc[:],
                        op=ALU.mult,
                    )

            # ---------------- local top-8 + value gather ----------------
            t8 = [smallp.tile([128, K], F32, tag=f"t8_{t}", name=f"t8_{t}") for t in range(NB)]
            i8 = [smallp.tile([128, K], U32, tag=f"i8_{t}", name=f"i8_{t}") for t in range(NB)]
            ag_in = dramp.tile([B, K], F32)
            for t in range(NB):
                nc.vector.max(out=t8[t][:], in_=scores_sb[t][:])
                nc.vector.max_index(out=i8[t][:], in_max=t8[t][:], in_values=scores_sb[t][:])
                nc.sync.dma_start(ag_in[t * 128 : (t + 1) * 128, :], t8[t][:])

            # issue all value gathers up front so they overlap the AllGather
            vg_tiles = [[None] * K for _ in range(NB)]
            for t in range(NB):
                for k in range(K):
                    vg = vgp.tile([128, D], F32, tag="vg", name=f"vg{t}_{k}")
                    nc.gpsimd.indirect_dma_start(
                        out=vg[:],
                        out_offset=None,
                        in_=vals[:],
                        in_offset=bass.IndirectOffsetOnAxis(ap=i8[t][:, k : k + 1], axis=0),
                    )
                    vg_tiles[t][k] = vg

            ag_out = dramp.tile([N_CORES * B, K], F32)
            nc.gpsimd.collective_compute(
                "AllGather",
                ALU.bypass,
                replica_groups=[list(range(N_CORES))],
                ins=[ag_in.opt()],
                outs=[ag_out.opt()],
            )

            # ---------------- global stage ----------------
            rs_in = dramp.tile([B, D], F32)
            for t in range(NB):
                g = smallp.tile([128, N_CORES, K], F32, tag="g")
                nc.sync.dma_start(
                    g[:],
                    ag_out[:].rearrange("(c t p) k -> t p c k", c=N_CORES, p=128)[t],
                )
                qv = qinv_col[:, t : t + 1]
                gf = g[:].rearrange("p c k -> p (c k)")
                nc.vector.tensor_scalar(
                    out=gf, in0=gf, scalar1=qv, scalar2=None, op0=ALU.mult
                )
                g8 = smallp.tile([128, K], F32, tag="g8")
                nc.vector.max(out=g8[:], in_=gf)
                m1 = g8[:, 0:1]
                thr = g8[:, K - 1 : K]
                negm1 = smallp.tile([128, 1], F32, tag="negm1")
                nc.vector.tensor_scalar(
                    out=negm1[:], in0=m1, scalar1=-1.0, scalar2=None, op0=ALU.mult
                )
                e8 = smallp.tile([128, K], F32, tag="e8")
                zsum = smallp.tile([128, 1], F32, tag="zsum")
                nc.scalar.activation(
                    out=e8[:], in_=g8[:], func=AF.Exp, bias=negm1[:], accum_out=zsum[:]
                )
                lnz = smallp.tile([128, 1], F32, tag="lnz")
                nc.scalar.activation(out=lnz[:], in_=zsum[:], func=AF.Ln)
                # confidence = 1/Z = exp(-lnz)
                cz = smallp.tile([128, 1], F32, tag="cz")
                nc.scalar.activation(out=cz[:], in_=lnz[:], func=AF.Exp, scale=-1.0)
                nc.sync.dma_start(conf[t * 128 : (t + 1) * 128].unsqueeze(-1), cz[:])

                # own candidate weights: w = exp(s*qinv - m1 - lnz) * (s*qinv >= thr)
                so = smallp.tile([128, K], F32, tag="so")
                nc.vector.tensor_scalar(
                    out=so[:], in0=t8[t][:], scalar1=qv, scalar2=None, op0=ALU.mult
                )
                msk = smallp.tile([128, K], F32, tag="msk")
                nc.vector.tensor_scalar(
                    out=msk[:], in0=so[:], scalar1=thr, scalar2=None, op0=ALU.is_ge
                )
                nbias = smallp.tile([128, 1], F32, tag="nbias")
                nc.vector.tensor_tensor(out=nbias[:], in0=negm1[:], in1=lnz[:], op=ALU.subtract)
                eo = smallp.tile([128, K], F32, tag="eo")
                nc.scalar.activation(out=eo[:], in_=so[:], func=AF.Exp, bias=nbias[:])
                wloc = smallp.tile([128, K], F32, tag="wloc")
                nc.vector.tensor_tensor(out=wloc[:], in0=eo[:], in1=msk[:], op=ALU.mult)

                # partial combine: pc = sum_k wloc[:, k] * v_k (fused mul-add chain)
                pc = pcp.tile([128, D], F32, tag="pc", name=f"pc{t}")
                for k in range(K):
                    if k == 0:
                        nc.vector.tensor_scalar(
                            out=pc[:], in0=vg_tiles[t][k][:], scalar1=wloc[:, 0:1],
                            scalar2=None, op0=ALU.mult,
                        )
                    else:
                        nc.vector.scalar_tensor_tensor(
                            out=pc[:], in0=vg_tiles[t][k][:], scalar=wloc[:, k : k + 1],
                            in1=pc[:], op0=ALU.mult, op1=ALU.add,
                        )
                nc.sync.dma_start(rs_in[t * 128 : (t + 1) * 128, :], pc[:])

            rs_out = dramp.tile([B // N_CORES, D], F32)
            nc.gpsimd.collective_compute(
                "ReduceScatter",
                ALU.add,
                replica_groups=[list(range(N_CORES))],
                ins=[rs_in.opt()],
                outs=[rs_out.opt()],
            )
            nc.sync.dma_start(comb[:], rs_out[:])

    nc.compile()
    return nc


_PROGRAM = None


def _get_program():
    global _PROGRAM
    if _PROGRAM is None:
        _PROGRAM = _build_program()
    return _PROGRAM


def run_on_hw(in_maps, trace=False):
    nc = _get_program()
    return run_bass_kernel_spmd(
        nc, in_maps, core_ids=list(range(N_CORES)), trace=trace
    )


def make_in_maps(query, mem_keys, mem_values, importance, access_times, access_counts):
    query = np.asarray(query, dtype=np.float32)
    mem_keys = np.asarray(mem_keys, dtype=np.float32)
    mem_values = np.asarray(mem_values, dtype=np.float32)
    importance = np.asarray(importance, dtype=np.float32)
    access_times = np.asarray(access_times, dtype=np.int32)
    access_counts = np.asarray(access_counts, dtype=np.int32)

    bf16 = ml_dtypes.bfloat16
    qT_np = np.ascontiguousarray(query.T)
    qh_np = qT_np.astype(bf16)
    ql_np = (qT_np - qh_np.astype(np.float32)).astype(bf16)

    in_maps = []
    for c in range(N_CORES):
        sl = slice(c * MC, (c + 1) * MC)
        kT_np = np.ascontiguousarray(mem_keys[sl].T)
        kh_np = kT_np.astype(bf16)
        kl_np = (kT_np - kh_np.astype(np.float32)).astype(bf16)
        ksq_np = (kT_np * kT_np).astype(np.float16)
        in_maps.append(
            {
                "qT": qT_np,
                "qh": qh_np,
                "ql": ql_np,
                "kh": kh_np,
                "kl": kl_np,
                "ksq": ksq_np,
                "vals": np.ascontiguousarray(mem_values[sl]),
                "imp": importance[sl],
                "at": access_times[sl],
                "cnt": access_counts[sl],
            }
        )
    return in_maps


def kernel(
    query,
    mem_keys,
    mem_values,
    importance,
    access_times,
    access_counts,
    current_time,
    top_k,
    _trace=False,
    _results_out=None,
):
    assert int(current_time) == 1000 and int(top_k) == 8
    in_maps = make_in_maps(
        query, mem_keys, mem_values, importance, access_times, access_counts
    )
    res = run_on_hw(in_maps, trace=_trace)
    if _results_out is not None:
        _results_out.append(res)
    combined = np.concatenate(
        [res.results[c]["comb"] for c in range(N_CORES)], axis=0
    )
    confidence = res.results[0]["conf"]
    return combined, confidence


# revision 8
# speedup vs baseline: 1.1893x; 1.0143x over previous
"""Distributed KNN retrieval kernel for Trainium2 (8 NeuronCores).

Strategy (standard distributed-KNN):
  - Shard the memory bank (mem_keys/mem_values + metadata) across 8 cores
    along the memory axis (8192 memories per core).
  - Each core: scores = (q @ k_shard.T) * (retention*importance*freq / ||k||)
    with the fp32 matmul decomposed into 3 bf16 matmuls (hi/lo split, done
    on the host as an input re-encoding; error ~2^-16 relative, far below
    the top-8 ranking margins), then hardware top-8 (InstMax/InstMaxIndex)
    per query, then indirect-DMA gather of its 8 candidate value rows.
  - AllGather the 8*8 candidate scores per query; every core computes the
    global top-8 threshold + softmax normalizer, weights its own surviving
    candidates, and emits a partial weighted combine.
  - ReduceScatter sums the partials; each core outputs a 32-query slice of
    the combined output. Host concatenates the slices.

Key norms use a pre-squared fp16 copy of the keys (ones-vector matmul
reduces over the contraction partitions), avoiding elementwise squares on
the vector engine. All transcendentals (decay exp, log1p, 1/sqrt via
exp(-0.5 ln), softmax) run on the scalar engine.
"""

import sys

for p in ("/opt/trn_rl_repo", "/opt/pypackages", "/root/.axon_site"):
    if p not in sys.path:
        sys.path.insert(0, p)

import math
import numpy as np
import ml_dtypes

import concourse.bass as bass
import concourse.bacc as bacc
import concourse.mybir as mybir
import concourse.tile as tile
from concourse.bass_utils import run_bass_kernel_spmd

N_CORES = 8
B = 256  # queries
D = 1024  # feature dim
M = 65536  # memory bank size
MC = M // N_CORES  # memories per core (8192)
K = 8  # top_k
NB = B // 128  # query partition tiles (2)
ND = D // 128  # contraction chunks (8)
MCHUNK = 512  # moving free dim per matmul
NCH = MC // MCHUNK  # m-chunks per core (16)
CUR_TIME = 1000.0
DECAY_RATE = 0.999
DECAY_EPS = 1e-8

F32 = mybir.dt.float32
F16 = mybir.dt.float16
BF16 = mybir.dt.bfloat16
I32 = mybir.dt.int32
U32 = mybir.dt.uint32
AF = mybir.ActivationFunctionType
ALU = mybir.AluOpType


def _build_program():
    nc = bacc.Bacc("TRN2", target_bir_lowering=False, debug=False, num_devices=N_CORES)

    # Per-core inputs
    qT = nc.dram_tensor("qT", [D, B], F32, kind="ExternalInput").ap()
    qh = nc.dram_tensor("qh", [D, B], BF16, kind="ExternalInput").ap()
    ql = nc.dram_tensor("ql", [D, B], BF16, kind="ExternalInput").ap()
    kh = nc.dram_tensor("kh", [D, MC], BF16, kind="ExternalInput").ap()
    kl = nc.dram_tensor("kl", [D, MC], BF16, kind="ExternalInput").ap()
    ksq = nc.dram_tensor("ksq", [D, MC], F16, kind="ExternalInput").ap()
    vals = nc.dram_tensor("vals", [MC, D], F32, kind="ExternalInput").ap()
    imp = nc.dram_tensor("imp", [MC], F32, kind="ExternalInput").ap()
    at = nc.dram_tensor("at", [MC], I32, kind="ExternalInput").ap()
    cnt = nc.dram_tensor("cnt", [MC], I32, kind="ExternalInput").ap()

    # Per-core outputs
    comb = nc.dram_tensor("comb", [B // N_CORES, D], F32, kind="ExternalOutput").ap()
    conf = nc.dram_tensor("conf", [B], F32, kind="ExternalOutput").ap()

    with tile.TileContext(nc) as tc:
        with (
            tc.tile_pool(name="const", bufs=1) as constp,
            tc.tile_pool(name="meta", bufs=1) as metap,
            tc.tile_pool(name="kt", bufs=2) as ktp,
            tc.tile_pool(name="acc", bufs=2) as accp,
            tc.tile_pool(name="wb", bufs=2) as wbp,
            tc.tile_pool(name="scores", bufs=1) as scoresp,
            tc.tile_pool(name="small", bufs=1) as smallp,
            tc.tile_pool(name="vg", bufs=10) as vgp,
            tc.tile_pool(name="pc", bufs=2) as pcp,
            tc.tile_pool(name="psum", bufs=4, space="PSUM") as psump,
            tc.tile_pool(name="psn", bufs=2, space="PSUM") as psnp,
            tc.tile_pool(name="dram", bufs=1, space="DRAM") as dramp,
        ):
            # ---------------- constants / query loads ----------------
            qh_sb = constp.tile([128, ND, B], BF16)  # [p, j, b] : d = 128*j + p
            ql_sb = constp.tile([128, ND, B], BF16)
            nc.sync.dma_start(qh_sb[:], qh.rearrange("(j p) b -> p j b", p=128))
            nc.sync.dma_start(ql_sb[:], ql.rearrange("(j p) b -> p j b", p=128))

            ones16 = constp.tile([128, 1], F16)
            nc.vector.memset(ones16[:], 1.0)

            qt_sb = constp.tile([128, ND, B], F32)
            nc.sync.dma_start(qt_sb[:], qT.rearrange("(j p) b -> p j b", p=128))

            # k-chunk loader (first two chunks issued before the metadata
            # DMAs so the PE ramp isn't blocked on them)
            kt_cache = {}

            def load_chunk(ci):
                kh_t = ktp.tile([128, ND, MCHUNK], BF16, tag="kh", name=f"kh_{ci}")
                kl_t = ktp.tile([128, ND, MCHUNK], BF16, tag="kl", name=f"kl_{ci}")
                ks_t = ktp.tile([128, ND, MCHUNK], F16, tag="ks", name=f"ks_{ci}")
                view = "(j p) (c n) -> c p j n"
                nc.sync.dma_start(kh_t[:], kh.rearrange(view, p=128, n=MCHUNK)[ci])
                nc.sync.dma_start(kl_t[:], kl.rearrange(view, p=128, n=MCHUNK)[ci])
                nc.sync.dma_start(ks_t[:], ksq.rearrange(view, p=128, n=MCHUNK)[ci])
                kt_cache[ci] = (kh_t, kl_t, ks_t)

            load_chunk(0)
            load_chunk(1)

            # ---------------- metadata -> meta (partition layout) ------------
            # m = 64*p + f
            imp_t = metap.tile([128, MC // 128], F32, tag="m_imp")
            at_i = metap.tile([128, MC // 128], I32, tag="m_at")
            cnt_i = metap.tile([128, MC // 128], I32, tag="m_cnt")
            nc.sync.dma_start(imp_t[:], imp.rearrange("(p f) -> p f", p=128))
            nc.sync.dma_start(at_i[:], at.rearrange("(p f) -> p f", p=128))
            nc.sync.dma_start(cnt_i[:], cnt.rearrange("(p f) -> p f", p=128))

            dt_f = metap.tile([128, MC // 128], F32, tag="m_dt")
            nc.vector.tensor_copy(out=dt_f[:], in_=at_i[:])  # i32 -> f32
            # dt = CUR_TIME - at  (as -1*at + CUR_TIME)
            nc.vector.tensor_scalar(
                out=dt_f[:], in0=dt_f[:], scalar1=-1.0, scalar2=CUR_TIME,
                op0=ALU.mult, op1=ALU.add,
            )
            ret_t = metap.tile([128, MC // 128], F32, tag="m_ret")
            nc.scalar.activation(
                out=ret_t[:], in_=dt_f[:], func=AF.Exp, scale=float(math.log(DECAY_RATE))
            )
            cnt_f = metap.tile([128, MC // 128], F32, tag="m_cntf")
            nc.vector.tensor_copy(out=cnt_f[:], in_=cnt_i[:])
            fb_t = metap.tile([128, MC // 128], F32, tag="m_fb")
            nc.scalar.activation(out=fb_t[:], in_=cnt_f[:], func=AF.Ln, bias=1.0)
            meta_t = metap.tile([128, MC // 128], F32, tag="m_meta")
            nc.vector.tensor_tensor(out=meta_t[:], in0=ret_t[:], in1=imp_t[:], op=ALU.mult)
            nc.vector.tensor_tensor(out=meta_t[:], in0=meta_t[:], in1=fb_t[:], op=ALU.mult)

            # bounce meta to DRAM for [1, chunk] row loads
            d_meta = dramp.tile([MC], F32)
            nc.sync.dma_start(d_meta[:].rearrange("(p f) -> p f", p=128), meta_t[:])

            # ---------------- query norms -> qinv columns ----------------
            accq = accp.tile([128, B], F32, tag="accq")
            sqq = accp.tile([128, B], F32, tag="sqq")
            for j in range(ND):
                tgt = accq if j == 0 else sqq
                nc.scalar.activation(out=tgt[:], in_=qt_sb[:, j, :], func=AF.Square)
                if j > 0:
                    nc.vector.tensor_tensor(out=accq[:], in0=accq[:], in1=sqq[:], op=ALU.add)
            ones32 = constp.tile([128, 1], F32)
            nc.vector.memset(ones32[:], 1.0)
            psq = psnp.tile([1, B], F32, tag="psq")
            nc.tensor.matmul(psq[:], ones32[:], accq[:], start=True, stop=True)
            # qinv = 1/sqrt(ssq) = exp(-0.5*ln(ssq))
            qn_row = smallp.tile([1, B], F32, tag="qn_row")
            nc.scalar.activation(out=qn_row[:], in_=psq[:], func=AF.Ln)
            nc.scalar.activation(out=qn_row[:], in_=qn_row[:], func=AF.Exp, scale=-0.5)
            d_qinv = dramp.tile([B], F32)
            nc.sync.dma_start(d_qinv[:].unsqueeze(0), qn_row[:])
            qinv_col = smallp.tile([128, NB], F32, tag="qinv_col")
            for t in range(NB):
                nc.sync.dma_start(
                    qinv_col[:, t : t + 1], d_qinv[t * 128 : (t + 1) * 128].unsqueeze(-1)
                )

            # ---------------- main loop: scores + k-norms ----------------
            scores_sb = [
                scoresp.tile([128, MC], F32, tag=f"sc{t}", name=f"scores{t}")
                for t in range(NB)
            ]

            q8 = [
                smallp.tile([128, 4 * K], F32, tag=f"q8_{t}", name=f"q8_{t}")
                for t in range(NB)
            ]

            for ci in range(NCH):
                if ci not in kt_cache:
                    load_chunk(ci)
                kh_t, kl_t, ks_t = kt_cache.pop(ci)

                # ||k||^2 via ones-matmul over partitions, accumulated over j
                psn = psnp.tile([1, MCHUNK], F32, tag="psn")
                for j in range(ND):
                    nc.tensor.matmul(
                        psn[:], ones16[:], ks_t[:, j, :],
                        start=(j == 0), stop=(j == ND - 1),
                    )

                # w = meta * exp(-0.5*ln(ss))
                mrow = wbp.tile([1, MCHUNK], F32, tag="mrow", name="mrow")
                nc.sync.dma_start(
                    mrow[:], d_meta[ci * MCHUNK : (ci + 1) * MCHUNK].unsqueeze(0)
                )
                wr = wbp.tile([1, MCHUNK], F32, tag="wr", name="wr")
                nc.scalar.activation(out=wr[:], in_=psn[:], func=AF.Ln)
                nc.scalar.activation(out=wr[:], in_=wr[:], func=AF.Exp, scale=-0.5)
                nc.vector.tensor_tensor(out=wr[:], in0=wr[:], in1=mrow[:], op=ALU.mult)
                w_bc = wbp.tile([128, MCHUNK], F32, tag="w_bc")
                nc.gpsimd.partition_broadcast(w_bc[:], wr[:])

                # scores matmuls: 3 bf16 groups accumulate into one psum tile
                for t in range(NB):
                    ps = psump.tile([128, MCHUNK], F32, tag="ps")
                    groups = [(qh_sb, kh_t), (qh_sb, kl_t), (ql_sb, kh_t)]
                    n_mm = len(groups) * ND
                    i_mm = 0
                    for qsb, ktile in groups:
                        for j in range(ND):
                            nc.tensor.matmul(
                                ps[:],
                                qsb[:, j, t * 128 : (t + 1) * 128],
                                ktile[:, j, :],
                                start=(i_mm == 0),
                                stop=(i_mm == n_mm - 1),
                            )
                            i_mm += 1
                    nc.vector.tensor_tensor(
                        out=scores_sb[t][:, ci * MCHUNK : (ci + 1) * MCHUNK],
                        in0=ps[:],
                        in1=w_bc[:],
                        op=ALU.mult,
                    )

                # per-quarter partial top-8 (hidden under the matmul phase)
                if ci % 4 == 3:
                    qi = ci // 4
                    for t in range(NB):
                        nc.vector.max(
                            out=q8[t][:, qi * K : (qi + 1) * K],
                            in_=scores_sb[t][:, (ci - 3) * MCHUNK : (ci + 1) * MCHUNK],
                        )

            # ---------------- local top-8: merge quarters, AG early ----------
            t8 = [smallp.tile([128, K], F32, tag=f"t8_{t}", name=f"t8_{t}") for t in range(NB)]
            i8 = [smallp.tile([128, K], U32, tag=f"i8_{t}", name=f"i8_{t}") for t in range(NB)]
            ag_in = dramp.tile([B, K], F32)
            for t in range(NB):
                nc.vector.max(out=t8[t][:], in_=q8[t][:])
                nc.sync.dma_start(ag_in[t * 128 : (t + 1) * 128, :], t8[t][:])

            ag_out = dramp.tile([N_CORES * B, K], F32)
            nc.gpsimd.collective_compute(
                "AllGather",
                ALU.bypass,
                replica_groups=[list(range(N_CORES))],
                ins=[ag_in.opt()],
                outs=[ag_out.opt()],
            )

            # indices + value gathers overlap the AllGather
            vg_tiles = [[None] * K for _ in range(NB)]
            for t in range(NB):
                nc.vector.max_index(out=i8[t][:], in_max=t8[t][:], in_values=scores_sb[t][:])
                for k in range(K):
                    vg = vgp.tile([128, D], F32, tag="vg", name=f"vg{t}_{k}")
                    nc.gpsimd.indirect_dma_start(
                        out=vg[:],
                        out_offset=None,
                        in_=vals[:],
                        in_offset=bass.IndirectOffsetOnAxis(ap=i8[t][:, k : k + 1], axis=0),
                    )
                    vg_tiles[t][k] = vg

            # ---------------- global stage (btiles interleaved, ACT funcs
            # batched to minimize activation-table reloads) ----------------
            g_t, g8_t, negm1_t, e8_t, zsum_t, lnz_t, cz_t = [], [], [], [], [], [], []
            so_t, msk_t, nbias_t, eo_t, wloc_t = [], [], [], [], []
            for t in range(NB):
                g = smallp.tile([128, N_CORES, K], F32, tag=f"g{t}", name=f"g{t}")
                nc.sync.dma_start(
                    g[:],
                    ag_out[:].rearrange("(c t p) k -> t p c k", c=N_CORES, p=128)[t],
                )
                g_t.append(g)
            for t in range(NB):
                qv = qinv_col[:, t : t + 1]
                gf = g_t[t][:].rearrange("p c k -> p (c k)")
                nc.vector.tensor_scalar(
                    out=gf, in0=gf, scalar1=qv, scalar2=None, op0=ALU.mult
                )
                g8 = smallp.tile([128, K], F32, tag=f"g8{t}", name=f"g8{t}")
                nc.vector.max(out=g8[:], in_=gf)
                g8_t.append(g8)
                negm1 = smallp.tile([128, 1], F32, tag=f"negm1{t}", name=f"negm1{t}")
                nc.vector.tensor_scalar(
                    out=negm1[:], in0=g8[:, 0:1], scalar1=-1.0, scalar2=None, op0=ALU.mult
                )
                negm1_t.append(negm1)
                so = smallp.tile([128, K], F32, tag=f"so{t}", name=f"so{t}")
                nc.vector.tensor_scalar(
                    out=so[:], in0=t8[t][:], scalar1=qv, scalar2=None, op0=ALU.mult
                )
                so_t.append(so)
                msk = smallp.tile([128, K], F32, tag=f"msk{t}", name=f"msk{t}")
                nc.vector.tensor_scalar(
                    out=msk[:], in0=so[:], scalar1=g8[:, K - 1 : K], scalar2=None,
                    op0=ALU.is_ge,
                )
                msk_t.append(msk)
            for t in range(NB):  # ACT: Exp
                e8 = smallp.tile([128, K], F32, tag=f"e8{t}", name=f"e8{t}")
                zsum = smallp.tile([128, 1], F32, tag=f"zsum{t}", name=f"zsum{t}")
                nc.scalar.activation(
                    out=e8[:], in_=g8_t[t][:], func=AF.Exp, bias=negm1_t[t][:],
                    accum_out=zsum[:],
                )
                e8_t.append(e8)
                zsum_t.append(zsum)
            for t in range(NB):  # ACT: Ln
                lnz = smallp.tile([128, 1], F32, tag=f"lnz{t}", name=f"lnz{t}")
                nc.scalar.activation(out=lnz[:], in_=zsum_t[t][:], func=AF.Ln)
                lnz_t.append(lnz)
                nbias = smallp.tile([128, 1], F32, tag=f"nbias{t}", name=f"nbias{t}")
                nc.vector.tensor_tensor(
                    out=nbias[:], in0=negm1_t[t][:], in1=lnz[:], op=ALU.subtract
                )
                nbias_t.append(nbias)
            for t in range(NB):  # ACT: Exp (confidence + own weights)
                cz = smallp.tile([128, 1], F32, tag=f"cz{t}", name=f"cz{t}")
                nc.scalar.activation(out=cz[:], in_=lnz_t[t][:], func=AF.Exp, scale=-1.0)
                nc.sync.dma_start(conf[t * 128 : (t + 1) * 128].unsqueeze(-1), cz[:])
                eo = smallp.tile([128, K], F32, tag=f"eo{t}", name=f"eo{t}")
                nc.scalar.activation(out=eo[:], in_=so_t[t][:], func=AF.Exp, bias=nbias_t[t][:])
                eo_t.append(eo)
            for t in range(NB):
                wloc = smallp.tile([128, K], F32, tag=f"wloc{t}", name=f"wloc{t}")
                nc.vector.tensor_tensor(out=wloc[:], in0=eo_t[t][:], in1=msk_t[t][:], op=ALU.mult)
                wloc_t.append(wloc)

            # partial combine + per-btile ReduceScatter (second overlaps first)
            rs_in = [dramp.tile([128, D], F32, name=f"rs_in{t}") for t in range(NB)]
            rs_out = [
                dramp.tile([128 // N_CORES, D], F32, name=f"rs_out{t}") for t in range(NB)
            ]
            for t in range(NB):
                wloc = wloc_t[t]
                pc = pcp.tile([128, D], F32, tag="pc", name=f"pc{t}")
                for k in range(K):
                    if k == 0:
                        nc.vector.tensor_scalar(
                            out=pc[:], in0=vg_tiles[t][k][:], scalar1=wloc[:, 0:1],
                            scalar2=None, op0=ALU.mult,
                        )
                    else:
                        nc.vector.scalar_tensor_tensor(
                            out=pc[:], in0=vg_tiles[t][k][:], scalar=wloc[:, k : k + 1],
                            in1=pc[:], op0=ALU.mult, op1=ALU.add,
                        )
                nc.sync.dma_start(rs_in[t][:], pc[:])
                nc.gpsimd.collective_compute(
                    "ReduceScatter",
                    ALU.add,
                    replica_groups=[list(range(N_CORES))],
                    ins=[rs_in[t].opt()],
                    outs=[rs_out[t].opt()],
                )
                nc.sync.dma_start(
                    comb[t * (128 // N_CORES) : (t + 1) * (128 // N_CORES), :],
                    rs_out[t][:],
                )

    nc.compile()
    return nc


_PROGRAM = None


def _get_program():
    global _PROGRAM
    if _PROGRAM is None:
        _PROGRAM = _build_program()
    return _PROGRAM


def run_on_hw(in_maps, trace=False):
    nc = _get_program()
    return run_bass_kernel_spmd(
        nc, in_maps, core_ids=list(range(N_CORES)), trace=trace
    )


def make_in_maps(query, mem_keys, mem_values, importance, access_times, access_counts):
    query = np.asarray(query, dtype=np.float32)
    mem_keys = np.asarray(mem_keys, dtype=np.float32)
    mem_values = np.asarray(mem_values, dtype=np.float32)
    importance = np.asarray(importance, dtype=np.float32)
    access_times = np.asarray(access_times, dtype=np.int32)
    access_counts = np.asarray(access_counts, dtype=np.int32)

    bf16 = ml_dtypes.bfloat16
    qT_np = np.ascontiguousarray(query.T)
    qh_np = qT_np.astype(bf16)
    ql_np = (qT_np - qh_np.astype(np.float32)).astype(bf16)

    in_maps = []
    for c in range(N_CORES):
        sl = slice(c * MC, (c + 1) * MC)
        kT_np = np.ascontiguousarray(mem_keys[sl].T)
        kh_np = kT_np.astype(bf16)
        kl_np = (kT_np - kh_np.astype(np.float32)).astype(bf16)
        ksq_np = (kT_np * kT_np).astype(np.float16)
        in_maps.append(
            {
                "qT": qT_np,
                "qh": qh_np,
                "ql": ql_np,
                "kh": kh_np,
                "kl": kl_np,
                "ksq": ksq_np,
                "vals": np.ascontiguousarray(mem_values[sl]),
                "imp": importance[sl],
                "at": access_times[sl],
                "cnt": access_counts[sl],
            }
        )
    return in_maps


def kernel(
    query,
    mem_keys,
    mem_values,
    importance,
    access_times,
    access_counts,
    current_time,
    top_k,
    _trace=False,
    _results_out=None,
):
    assert int(current_time) == 1000 and int(top_k) == 8
    in_maps = make_in_maps(
        query, mem_keys, mem_values, importance, access_times, access_counts
    )
    res = run_on_hw(in_maps, trace=_trace)
    if _results_out is not None:
        _results_out.append(res)
    S = 128 // N_CORES  # rows per core per btile shard (16)
    combined = np.empty((B, D), dtype=np.float32)
    for c in range(N_CORES):
        cb = res.results[c]["comb"]
        combined[S * c : S * (c + 1)] = cb[:S]
        combined[128 + S * c : 128 + S * (c + 1)] = cb[S:]
    confidence = res.results[0]["conf"]
    return combined, confidence


# revision 10
# speedup vs baseline: 1.2825x; 1.0783x over previous
"""Distributed KNN retrieval kernel for Trainium2 (8 NeuronCores).

Strategy (standard distributed-KNN):
  - Shard the memory bank (mem_keys/mem_values + metadata) across 8 cores
    along the memory axis (8192 memories per core).
  - Each core: scores = (q @ k_shard.T) * (retention*importance*freq / ||k||)
    with the fp32 matmul decomposed into 3 bf16 matmuls (hi/lo split, done
    on the host as an input re-encoding; error ~2^-16 relative, far below
    the top-8 ranking margins), then hardware top-8 (InstMax/InstMaxIndex)
    per query, then indirect-DMA gather of its 8 candidate value rows.
  - AllGather the 8*8 candidate scores per query; every core computes the
    global top-8 threshold + softmax normalizer, weights its own surviving
    candidates, and emits a partial weighted combine.
  - ReduceScatter sums the partials; each core outputs a 32-query slice of
    the combined output. Host concatenates the slices.

Key norms use a pre-squared fp16 copy of the keys (ones-vector matmul
reduces over the contraction partitions), avoiding elementwise squares on
the vector engine. All transcendentals (decay exp, log1p, 1/sqrt via
exp(-0.5 ln), softmax) run on the scalar engine.
"""

import sys

for p in ("/opt/trn_rl_repo", "/opt/pypackages", "/root/.axon_site"):
    if p not in sys.path:
        sys.path.insert(0, p)

import math
import numpy as np
import ml_dtypes

import concourse.bass as bass
from concourse.bass import _add_dep_helper
import concourse.bacc as bacc
import concourse.mybir as mybir
import concourse.tile as tile
from concourse.bass_utils import run_bass_kernel_spmd

N_CORES = 8
B = 256  # queries
D = 1024  # feature dim
M = 65536  # memory bank size
MC = M // N_CORES  # memories per core (8192)
K = 8  # top_k
NB = B // 128  # query partition tiles (2)
ND = D // 128  # contraction chunks (8)
MCHUNK = 512  # moving free dim per matmul
NCH = MC // MCHUNK  # m-chunks per core (16)
CUR_TIME = 1000.0
DECAY_RATE = 0.999
DECAY_EPS = 1e-8

F32 = mybir.dt.float32
F16 = mybir.dt.float16
BF16 = mybir.dt.bfloat16
I32 = mybir.dt.int32
U32 = mybir.dt.uint32
AF = mybir.ActivationFunctionType
ALU = mybir.AluOpType


def _build_program():
    nc = bacc.Bacc("TRN2", target_bir_lowering=False, debug=False, num_devices=N_CORES)

    # Per-core inputs
    qh = nc.dram_tensor("qh", [D, B], BF16, kind="ExternalInput").ap()
    ql = nc.dram_tensor("ql", [D, B], BF16, kind="ExternalInput").ap()
    kh = nc.dram_tensor("kh", [D, MC], BF16, kind="ExternalInput").ap()
    kl = nc.dram_tensor("kl", [D, MC], BF16, kind="ExternalInput").ap()
    ksq = nc.dram_tensor("ksq", [D, MC], F16, kind="ExternalInput").ap()
    vals = nc.dram_tensor("vals", [MC, D], F32, kind="ExternalInput").ap()
    imp = nc.dram_tensor("imp", [MC], F32, kind="ExternalInput").ap()
    at = nc.dram_tensor("at", [MC], I32, kind="ExternalInput").ap()
    cnt = nc.dram_tensor("cnt", [MC], I32, kind="ExternalInput").ap()

    # Per-core outputs
    comb = nc.dram_tensor("comb", [B // N_CORES, D], F32, kind="ExternalOutput").ap()
    conf = nc.dram_tensor("conf", [B], F32, kind="ExternalOutput").ap()

    with tile.TileContext(nc) as tc:
        with (
            tc.tile_pool(name="const", bufs=1) as constp,
            tc.tile_pool(name="meta", bufs=1) as metap,
            tc.tile_pool(name="kt", bufs=2) as ktp,
            tc.tile_pool(name="acc", bufs=2) as accp,
            tc.tile_pool(name="wb", bufs=2) as wbp,
            tc.tile_pool(name="scores", bufs=1) as scoresp,
            tc.tile_pool(name="small", bufs=1) as smallp,
            tc.tile_pool(name="vg", bufs=12) as vgp,
            tc.tile_pool(name="pc", bufs=2) as pcp,
            tc.tile_pool(name="psum", bufs=4, space="PSUM") as psump,
            tc.tile_pool(name="psn", bufs=2, space="PSUM") as psnp,
            tc.tile_pool(name="dram", bufs=1, space="DRAM") as dramp,
        ):
            # ---------------- constants / query loads ----------------
            qh_sb = constp.tile([128, ND, B], BF16)  # [p, j, b] : d = 128*j + p
            ql_sb = constp.tile([128, ND, B], BF16)
            nc.sync.dma_start(qh_sb[:], qh.rearrange("(j p) b -> p j b", p=128))
            nc.sync.dma_start(ql_sb[:], ql.rearrange("(j p) b -> p j b", p=128))

            ones16 = constp.tile([128, 1], F16)
            nc.vector.memset(ones16[:], 1.0)


            # k-chunk loader (first two chunks issued before the metadata
            # DMAs so the PE ramp isn't blocked on them)
            kt_cache = {}

            def load_chunk(ci):
                kh_t = ktp.tile([128, ND, MCHUNK], BF16, tag="kh", name=f"kh_{ci}")
                kl_t = ktp.tile([128, ND, MCHUNK], BF16, tag="kl", name=f"kl_{ci}")
                ks_t = ktp.tile([128, ND, MCHUNK], F16, tag="ks", name=f"ks_{ci}")
                view = "(j p) (c n) -> c p j n"
                nc.sync.dma_start(kh_t[:], kh.rearrange(view, p=128, n=MCHUNK)[ci])
                nc.sync.dma_start(kl_t[:], kl.rearrange(view, p=128, n=MCHUNK)[ci])
                nc.sync.dma_start(ks_t[:], ksq.rearrange(view, p=128, n=MCHUNK)[ci])
                kt_cache[ci] = (kh_t, kl_t, ks_t)

            load_chunk(0)
            load_chunk(1)

            # ---------------- metadata -> meta (partition layout) ------------
            # m = 64*p + f
            imp_t = metap.tile([128, MC // 128], F32, tag="m_imp")
            at_i = metap.tile([128, MC // 128], I32, tag="m_at")
            cnt_i = metap.tile([128, MC // 128], I32, tag="m_cnt")
            nc.sync.dma_start(imp_t[:], imp.rearrange("(p f) -> p f", p=128))
            nc.sync.dma_start(at_i[:], at.rearrange("(p f) -> p f", p=128))
            nc.sync.dma_start(cnt_i[:], cnt.rearrange("(p f) -> p f", p=128))

            dt_f = metap.tile([128, MC // 128], F32, tag="m_dt")
            nc.vector.tensor_copy(out=dt_f[:], in_=at_i[:])  # i32 -> f32
            # dt = CUR_TIME - at  (as -1*at + CUR_TIME)
            nc.vector.tensor_scalar(
                out=dt_f[:], in0=dt_f[:], scalar1=-1.0, scalar2=CUR_TIME,
                op0=ALU.mult, op1=ALU.add,
            )
            ret_t = metap.tile([128, MC // 128], F32, tag="m_ret")
            nc.scalar.activation(
                out=ret_t[:], in_=dt_f[:], func=AF.Exp, scale=float(math.log(DECAY_RATE))
            )
            cnt_f = metap.tile([128, MC // 128], F32, tag="m_cntf")
            nc.vector.tensor_copy(out=cnt_f[:], in_=cnt_i[:])
            fb_t = metap.tile([128, MC // 128], F32, tag="m_fb")
            nc.scalar.activation(out=fb_t[:], in_=cnt_f[:], func=AF.Ln, bias=1.0)
            meta_t = metap.tile([128, MC // 128], F32, tag="m_meta")
            nc.vector.tensor_tensor(out=meta_t[:], in0=ret_t[:], in1=imp_t[:], op=ALU.mult)
            nc.vector.tensor_tensor(out=meta_t[:], in0=meta_t[:], in1=fb_t[:], op=ALU.mult)

            # bounce meta to DRAM for [1, chunk] row loads
            d_meta = dramp.tile([MC], F32)
            nc.sync.dma_start(d_meta[:].rearrange("(p f) -> p f", p=128), meta_t[:])

            # ---------------- query norms -> qinv columns ----------------
            accq = accp.tile([128, B], F32, tag="accq")
            sqq = accp.tile([128, B], F32, tag="sqq")
            qf = accp.tile([128, B], F32, tag="qf")
            for j in range(ND):
                nc.vector.tensor_tensor(
                    out=qf[:], in0=qh_sb[:, j, :], in1=ql_sb[:, j, :], op=ALU.add
                )
                tgt = accq if j == 0 else sqq
                nc.scalar.activation(out=tgt[:], in_=qf[:], func=AF.Square)
                if j > 0:
                    nc.vector.tensor_tensor(out=accq[:], in0=accq[:], in1=sqq[:], op=ALU.add)
            ones32 = constp.tile([128, 1], F32)
            nc.vector.memset(ones32[:], 1.0)
            psq = psnp.tile([1, B], F32, tag="psq")
            nc.tensor.matmul(psq[:], ones32[:], accq[:], start=True, stop=True)
            # qinv = 1/sqrt(ssq) = exp(-0.5*ln(ssq))
            qn_row = smallp.tile([1, B], F32, tag="qn_row")
            nc.scalar.activation(out=qn_row[:], in_=psq[:], func=AF.Ln)
            nc.scalar.activation(out=qn_row[:], in_=qn_row[:], func=AF.Exp, scale=-0.5)
            d_qinv = dramp.tile([B], F32)
            nc.sync.dma_start(d_qinv[:].unsqueeze(0), qn_row[:])
            qinv_col = smallp.tile([128, NB], F32, tag="qinv_col")
            for t in range(NB):
                nc.sync.dma_start(
                    qinv_col[:, t : t + 1], d_qinv[t * 128 : (t + 1) * 128].unsqueeze(-1)
                )

            # ---------------- main loop: scores + k-norms ----------------
            scores_sb = [
                scoresp.tile([128, MC], F32, tag=f"sc{t}", name=f"scores{t}")
                for t in range(NB)
            ]

            q8 = [
                smallp.tile([128, 4 * K], F32, tag=f"q8_{t}", name=f"q8_{t}")
                for t in range(NB)
            ]

            for ci in range(NCH):
                if ci not in kt_cache:
                    load_chunk(ci)
                kh_t, kl_t, ks_t = kt_cache.pop(ci)

                # ||k||^2 via ones-matmul over partitions, accumulated over j
                psn = psnp.tile([1, MCHUNK], F32, tag="psn")
                for j in range(ND):
                    nc.tensor.matmul(
                        psn[:], ones16[:], ks_t[:, j, :],
                        start=(j == 0), stop=(j == ND - 1),
                    )

                # w = meta * exp(-0.5*ln(ss))
                mrow = wbp.tile([1, MCHUNK], F32, tag="mrow", name="mrow")
                nc.sync.dma_start(
                    mrow[:], d_meta[ci * MCHUNK : (ci + 1) * MCHUNK].unsqueeze(0)
                )
                wr = wbp.tile([1, MCHUNK], F32, tag="wr", name="wr")
                nc.scalar.activation(out=wr[:], in_=psn[:], func=AF.Ln)
                nc.scalar.activation(out=wr[:], in_=wr[:], func=AF.Exp, scale=-0.5)
                nc.vector.tensor_tensor(out=wr[:], in0=wr[:], in1=mrow[:], op=ALU.mult)
                w_bc = wbp.tile([128, MCHUNK], F32, tag="w_bc")
                nc.gpsimd.partition_broadcast(w_bc[:], wr[:])

                # scores matmuls: 3 bf16 groups accumulate into one psum tile
                for t in range(NB):
                    ps = psump.tile([128, MCHUNK], F32, tag="ps")
                    groups = [(qh_sb, kh_t), (qh_sb, kl_t), (ql_sb, kh_t)]
                    n_mm = len(groups) * ND
                    i_mm = 0
                    for qsb, ktile in groups:
                        for j in range(ND):
                            nc.tensor.matmul(
                                ps[:],
                                qsb[:, j, t * 128 : (t + 1) * 128],
                                ktile[:, j, :],
                                start=(i_mm == 0),
                                stop=(i_mm == n_mm - 1),
                            )
                            i_mm += 1
                    nc.vector.tensor_tensor(
                        out=scores_sb[t][:, ci * MCHUNK : (ci + 1) * MCHUNK],
                        in0=ps[:],
                        in1=w_bc[:],
                        op=ALU.mult,
                    )

                # per-quarter partial top-8 (hidden under the matmul phase)
                if ci % 4 == 3:
                    qi = ci // 4
                    for t in range(NB):
                        nc.vector.max(
                            out=q8[t][:, qi * K : (qi + 1) * K],
                            in_=scores_sb[t][:, (ci - 3) * MCHUNK : (ci + 1) * MCHUNK],
                        )

            # ---------------- local top-8: merge quarters, AG early ----------
            t8 = [smallp.tile([128, K], F32, tag=f"t8_{t}", name=f"t8_{t}") for t in range(NB)]
            i8 = [smallp.tile([128, K], U32, tag=f"i8_{t}", name=f"i8_{t}") for t in range(NB)]
            ag_in = dramp.tile([B, K], F32)
            merge_insts = []
            for t in range(NB):
                merge_insts.append(nc.vector.max(out=t8[t][:], in_=q8[t][:]))
                nc.sync.dma_start(ag_in[t * 128 : (t + 1) * 128, :], t8[t][:])

            ag_out = dramp.tile([N_CORES * B, K], F32)
            nc.gpsimd.collective_compute(
                "AllGather",
                ALU.bypass,
                replica_groups=[list(range(N_CORES))],
                ins=[ag_in.opt()],
                outs=[ag_out.opt()],
            )

            # indices + value gathers overlap the AllGather
            vg_tiles = [[None] * K for _ in range(NB)]
            for t in range(NB):
                mi = nc.vector.max_index(
                    out=i8[t][:], in_max=t8[t][:], in_values=scores_sb[t][:]
                )
                for m in merge_insts:
                    _add_dep_helper(mi.ins, m.ins, sync=False,
                                    reason="AG input before index scan")
                for k in range(K):
                    vg = vgp.tile([128, D], F32, tag="vg", name=f"vg{t}_{k}")
                    nc.gpsimd.indirect_dma_start(
                        out=vg[:],
                        out_offset=None,
                        in_=vals[:],
                        in_offset=bass.IndirectOffsetOnAxis(ap=i8[t][:, k : k + 1], axis=0),
                    )
                    vg_tiles[t][k] = vg

            # ---------------- global stage (btiles interleaved, ACT funcs
            # batched to minimize activation-table reloads) ----------------
            g_t, g8_t, negm1_t, e8_t, zsum_t, lnz_t, cz_t = [], [], [], [], [], [], []
            so_t, msk_t, nbias_t, eo_t, wloc_t = [], [], [], [], []
            for t in range(NB):
                g = smallp.tile([128, N_CORES, K], F32, tag=f"g{t}", name=f"g{t}")
                nc.sync.dma_start(
                    g[:],
                    ag_out[:].rearrange("(c t p) k -> t p c k", c=N_CORES, p=128)[t],
                )
                g_t.append(g)
            for t in range(NB):
                qv = qinv_col[:, t : t + 1]
                gf = g_t[t][:].rearrange("p c k -> p (c k)")
                nc.vector.tensor_scalar(
                    out=gf, in0=gf, scalar1=qv, scalar2=None, op0=ALU.mult
                )
                g8 = smallp.tile([128, K], F32, tag=f"g8{t}", name=f"g8{t}")
                nc.vector.max(out=g8[:], in_=gf)
                g8_t.append(g8)
                negm1 = smallp.tile([128, 1], F32, tag=f"negm1{t}", name=f"negm1{t}")
                nc.vector.tensor_scalar(
                    out=negm1[:], in0=g8[:, 0:1], scalar1=-1.0, scalar2=None, op0=ALU.mult
                )
                negm1_t.append(negm1)
                so = smallp.tile([128, K], F32, tag=f"so{t}", name=f"so{t}")
                nc.vector.tensor_scalar(
                    out=so[:], in0=t8[t][:], scalar1=qv, scalar2=None, op0=ALU.mult
                )
                so_t.append(so)
                msk = smallp.tile([128, K], F32, tag=f"msk{t}", name=f"msk{t}")
                nc.vector.tensor_scalar(
                    out=msk[:], in0=so[:], scalar1=g8[:, K - 1 : K], scalar2=None,
                    op0=ALU.is_ge,
                )
                msk_t.append(msk)
            for t in range(NB):  # ACT: Exp
                e8 = smallp.tile([128, K], F32, tag=f"e8{t}", name=f"e8{t}")
                zsum = smallp.tile([128, 1], F32, tag=f"zsum{t}", name=f"zsum{t}")
                nc.scalar.activation(
                    out=e8[:], in_=g8_t[t][:], func=AF.Exp, bias=negm1_t[t][:],
                    accum_out=zsum[:],
                )
                e8_t.append(e8)
                zsum_t.append(zsum)
            for t in range(NB):  # ACT: Ln
                lnz = smallp.tile([128, 1], F32, tag=f"lnz{t}", name=f"lnz{t}")
                nc.scalar.activation(out=lnz[:], in_=zsum_t[t][:], func=AF.Ln)
                lnz_t.append(lnz)
                nbias = smallp.tile([128, 1], F32, tag=f"nbias{t}", name=f"nbias{t}")
                nc.vector.tensor_tensor(
                    out=nbias[:], in0=negm1_t[t][:], in1=lnz[:], op=ALU.subtract
                )
                nbias_t.append(nbias)
            for t in range(NB):  # ACT: Exp (confidence + own weights)
                cz = smallp.tile([128, 1], F32, tag=f"cz{t}", name=f"cz{t}")
                nc.scalar.activation(out=cz[:], in_=lnz_t[t][:], func=AF.Exp, scale=-1.0)
                nc.sync.dma_start(conf[t * 128 : (t + 1) * 128].unsqueeze(-1), cz[:])
                eo = smallp.tile([128, K], F32, tag=f"eo{t}", name=f"eo{t}")
                nc.scalar.activation(out=eo[:], in_=so_t[t][:], func=AF.Exp, bias=nbias_t[t][:])
                eo_t.append(eo)
            for t in range(NB):
                wloc = smallp.tile([128, K], F32, tag=f"wloc{t}", name=f"wloc{t}")
                nc.vector.tensor_tensor(out=wloc[:], in0=eo_t[t][:], in1=msk_t[t][:], op=ALU.mult)
                wloc_t.append(wloc)

            # partial combine + single ReduceScatter
            rs_in = dramp.tile([B, D], F32)
            for t in range(NB):
                wloc = wloc_t[t]
                pc = pcp.tile([128, D], F32, tag="pc", name=f"pc{t}")
                for k in range(K):
                    if k == 0:
                        nc.vector.tensor_scalar(
                            out=pc[:], in0=vg_tiles[t][k][:], scalar1=wloc[:, 0:1],
                            scalar2=None, op0=ALU.mult,
                        )
                    else:
                        nc.vector.scalar_tensor_tensor(
                            out=pc[:], in0=vg_tiles[t][k][:], scalar=wloc[:, k : k + 1],
                            in1=pc[:], op0=ALU.mult, op1=ALU.add,
                        )
                nc.sync.dma_start(rs_in[t * 128 : (t + 1) * 128, :], pc[:])
            rs_out = dramp.tile([B // N_CORES, D], F32)
            nc.gpsimd.collective_compute(
                "ReduceScatter",
                ALU.add,
                replica_groups=[list(range(N_CORES))],
                ins=[rs_in.opt()],
                outs=[rs_out.opt()],
            )
            nc.sync.dma_start(comb[:], rs_out[:])

    nc.compile()
    return nc


_PROGRAM = None


def _get_program():
    global _PROGRAM
    if _PROGRAM is None:
        _PROGRAM = _build_program()
    return _PROGRAM


def run_on_hw(in_maps, trace=False):
    nc = _get_program()
    return run_bass_kernel_spmd(
        nc, in_maps, core_ids=list(range(N_CORES)), trace=trace
    )


def make_in_maps(query, mem_keys, mem_values, importance, access_times, access_counts):
    query = np.asarray(query, dtype=np.float32)
    mem_keys = np.asarray(mem_keys, dtype=np.float32)
    mem_values = np.asarray(mem_values, dtype=np.float32)
    importance = np.asarray(importance, dtype=np.float32)
    access_times = np.asarray(access_times, dtype=np.int32)
    access_counts = np.asarray(access_counts, dtype=np.int32)

    bf16 = ml_dtypes.bfloat16
    qT_np = np.ascontiguousarray(query.T)
    qh_np = qT_np.astype(bf16)
    ql_np = (qT_np - qh_np.astype(np.float32)).astype(bf16)
    del qT_np

    in_maps = []
    for c in range(N_CORES):
        sl = slice(c * MC, (c + 1) * MC)
        kT_np = np.ascontiguousarray(mem_keys[sl].T)
        kh_np = kT_np.astype(bf16)
        kl_np = (kT_np - kh_np.astype(np.float32)).astype(bf16)
        ksq_np = (kT_np * kT_np).astype(np.float16)
        in_maps.append(
            {
                "qh": qh_np,
                "ql": ql_np,
                "kh": kh_np,
                "kl": kl_np,
                "ksq": ksq_np,
                "vals": np.ascontiguousarray(mem_values[sl]),
                "imp": importance[sl],
                "at": access_times[sl],
                "cnt": access_counts[sl],
            }
        )
    return in_maps


def kernel(
    query,
    mem_keys,
    mem_values,
    importance,
    access_times,
    access_counts,
    current_time,
    top_k,
    _trace=False,
    _results_out=None,
):
    assert int(current_time) == 1000 and int(top_k) == 8
    in_maps = make_in_maps(
        query, mem_keys, mem_values, importance, access_times, access_counts
    )
    res = run_on_hw(in_maps, trace=_trace)
    if _results_out is not None:
        _results_out.append(res)
    combined = np.concatenate(
        [res.results[c]["comb"] for c in range(N_CORES)], axis=0
    )
    confidence = res.results[0]["conf"]
    return combined, confidence
